# revision 17
# baseline (speedup 1.0000x reference)
"""Trainium2 Bass kernel for a 2-layer GAT (PyG GATConv semantics).

Strategy (8 NeuronCores, SPMD):
  - Host relabels nodes: dsts dealt to 8 cores snake-by-in-degree, grouped
    into 98 blocks of 128 dsts per core (block edge-counts equalized).
  - Edges (incl. self-loops) are dst-sorted per core and padded so every
    block owns exactly n_bt tiles of 128 edge slots -> one uniform SPMD
    program for all cores.
  - Launch A (dense): featT = W1ext.T @ xT per core shard. W1ext packs
    W1 plus per-head attention columns, so als/ald (and 0.2x copies) come
    out of the same matmul, fp32.
  - Host gathers per-edge streams (pure data movement): hd[src] as bf16,
    (als[src], ald[dst], 0.2 als[src], 0.2 ald[dst]) as fp32.
  - Launch B (L1 edge phase): per block: batched z = als+ald, leaky via
    max(z, 0.2z), exp on ACT (expanded per-head), hd_s = hd * ex, onehot
    (is_equal vs iota) per tile, PE matmuls accumulate agg/den in PSUM,
    epilogue normalizes + relu -> h, transposes and applies W2ext to
    produce (h2d | als2 | ald2) per node.
  - Host gathers L2 per-edge streams; Launch C = L2 edge phase -> out2.
All FLOPs happen on device; the host only permutes/gathers/casts.
"""

import os
import numpy as np
import ml_dtypes

N_NODES = 100000
N_EDGES = 1600000
IN_DIM = 128
HID = 128
HEADS = 4
C1 = 32
OUT_DIM = 64
NEG = 0.2
NC = 8
NODES_PER_CORE = 12544  # 98 blocks * 128
N_BLOCKS = 98
REAL_PER_CORE = 12500
N_PAD = NC * NODES_PER_CORE

BF16 = ml_dtypes.bfloat16

_cache = {}


# ----------------------------------------------------------------------------
# Host-side graph preparation (indexing only)
# ----------------------------------------------------------------------------

def _prep(edge_index):
    src0 = np.asarray(edge_index[0], dtype=np.int64)
    dst0 = np.asarray(edge_index[1], dtype=np.int64)
    loop = np.arange(N_NODES, dtype=np.int64)
    src = np.concatenate([src0, loop]).astype(np.int32)
    dst = np.concatenate([dst0, loop]).astype(np.int32)
    E = src.shape[0]

    deg = np.bincount(dst, minlength=N_NODES)
    order = np.argsort(-deg, kind="stable")  # nodes by in-degree desc

    # snake-deal nodes to cores
    i = np.arange(N_NODES)
    r, j = i // NC, i % NC
    core_of_rank = np.where(r % 2 == 0, j, NC - 1 - j)
    # rank within core
    rank_in_core = np.zeros(N_NODES, dtype=np.int64)
    for c in range(NC):
        m = core_of_rank == c
        rank_in_core[m] = np.arange(m.sum())
    # snake-deal a core's nodes into 98 blocks (equalizes block edge sums)
    k = rank_in_core
    rb, jb = k // N_BLOCKS, k % N_BLOCKS
    block_of = np.where(rb % 2 == 0, jb, N_BLOCKS - 1 - jb)
    slot_of = rb  # < 128 since 12500/98 < 128

    new_id = np.empty(N_NODES, dtype=np.int64)
    new_id[order] = core_of_rank * NODES_PER_CORE + block_of * 128 + slot_of
    old_of_new = np.full(N_PAD, -1, dtype=np.int64)
    old_of_new[new_id] = np.arange(N_NODES)

    s_new = new_id[src]
    d_new = new_id[dst]
    core_e = d_new // NODES_PER_CORE

    cores = []
    max_bt = 0
    for c in range(NC):
        m = core_e == c
        sc, dc = s_new[m], d_new[m]
        o = np.argsort(dc, kind="stable")
        sc, dc = sc[o], dc[o]
        dloc = dc - c * NODES_PER_CORE
        blk = dloc // 128
        cnt = np.bincount(blk, minlength=N_BLOCKS)
        max_bt = max(max_bt, int(np.ceil(cnt.max() / 128)))
        cores.append((sc, dloc, blk, cnt))

    n_bt = max_bt
    NT = N_BLOCKS * n_bt
    E_pad = NT * 128

    src_g = np.zeros((NC, E_pad), dtype=np.int64)   # new-node id of edge src
    dst_g = np.zeros((NC, E_pad), dtype=np.int64)   # new-node id of edge dst
    dloc_g = np.full((NC, E_pad), 128.0, dtype=np.float32)  # sentinel 128
    valid = np.zeros((NC, E_pad), dtype=bool)
    for c in range(NC):
        sc, dloc, blk, cnt = cores[c]
        ofs = 0
        pos = np.empty(len(sc), dtype=np.int64)
        start = np.concatenate([[0], np.cumsum(cnt)[:-1]])
        for b in range(N_BLOCKS):
            sl = slice(start[b], start[b] + cnt[b])
            pos[sl] = b * n_bt * 128 + np.arange(cnt[b])
        src_g[c, pos] = sc
        dst_g[c, pos] = dloc + c * NODES_PER_CORE
        dloc_g[c, pos] = (dloc % 128).astype(np.float32)
        valid[c, pos] = True

    return dict(n_bt=n_bt, NT=NT, E_pad=E_pad, old_of_new=old_of_new,
                new_id=new_id, src_g=src_g, dst_g=dst_g, dloc_g=dloc_g,
                valid=valid)


def _w1ext(W1, att_src1, att_dst1):
    # [128, 144] fp32: W1 | asrc blockdiag | 0.2 asrc | adst | 0.2 adst
    W1 = np.asarray(W1, np.float32)
    a_s = np.asarray(att_src1, np.float32)
    a_d = np.asarray(att_dst1, np.float32)
    bs = np.zeros((IN_DIM, HEADS), np.float32)
    bd = np.zeros((IN_DIM, HEADS), np.float32)
    # als[n,h] = sum_c hd[n, 32h+c]*a_s[h,c] = x @ (W1 @ asrc_bd)
    asrc_bd = np.zeros((HID, HEADS), np.float32)
    adst_bd = np.zeros((HID, HEADS), np.float32)
    for h in range(HEADS):
        asrc_bd[32 * h:32 * h + 32, h] = a_s[h]
        adst_bd[32 * h:32 * h + 32, h] = a_d[h]
    ws = W1 @ asrc_bd
    wd = W1 @ adst_bd
    return np.concatenate([W1, ws, NEG * ws, wd, NEG * wd], axis=1)


def _w2ext(W2, att_src2, att_dst2):
    W2 = np.asarray(W2, np.float32)
    a2s = np.asarray(att_src2, np.float32).reshape(-1)
    a2d = np.asarray(att_dst2, np.float32).reshape(-1)
    ws = (W2 @ a2s)[:, None]
    wd = (W2 @ a2d)[:, None]
    return np.concatenate([W2, ws, wd], axis=1)  # [128, 66]


def _pmaj(arr, NT):
    # [E_pad, F] -> [N_BLOCKS, 128, n_bt, F]; edge (b, t, p) at [b, p, t]
    F = arr.shape[1] if arr.ndim == 2 else 1
    n_bt = NT // N_BLOCKS
    a = arr.reshape(N_BLOCKS, n_bt, 128, F)
    return np.ascontiguousarray(a.transpose(0, 2, 1, 3))


# ----------------------------------------------------------------------------
# numpy emulation of the device dataflow (for validation)
# ----------------------------------------------------------------------------

def _run_numpy(x, meta, W1e, W2e):
    n_bt, NT = meta["n_bt"], meta["NT"]
    xp = np.zeros((N_PAD, IN_DIM), np.float32)
    real = meta["old_of_new"] >= 0
    xp[real] = np.asarray(x, np.float32)[meta["old_of_new"][real]]

    # Launch A: featT per core
    feat = xp @ W1e  # [N_PAD, 144]
    hd_bf = feat[:, :128].astype(BF16)
    als, als2x = feat[:, 128:132], feat[:, 132:136]
    ald, ald2x = feat[:, 136:140], feat[:, 140:144]

    h2a = np.zeros((N_PAD, 66), np.float32)
    for c in range(NC):
        sg, dg = meta["src_g"][c], meta["dst_g"][c]
        v = meta["valid"][c]
        hdg = hd_bf[sg] * v[:, None]
        z1 = (als[sg] + ald[dg]) * v[:, None]
        z2 = (als2x[sg] + ald2x[dg]) * v[:, None]
        ex = np.exp(np.maximum(z1, z2)).astype(np.float32)  # [E,4]
        exx = np.repeat(ex, 32, axis=1).astype(BF16)
        hs = (hdg.astype(np.float32) * exx.astype(np.float32)).astype(BF16)
        dloc = meta["dloc_g"][c]
        for b in range(N_BLOCKS):
            sl = slice(b * n_bt * 128, (b + 1) * n_bt * 128)
            oh = (dloc[sl, None] == np.arange(128)[None, :])  # [Eb, 128]
            agg = oh.T.astype(np.float32) @ hs[sl].astype(np.float32)
            den = oh.T.astype(np.float32) @ ex[sl]
            with np.errstate(divide="ignore", invalid="ignore"):
                rden = 1.0 / den
            h = agg.reshape(128, 4, 32) * rden[:, :, None]
            h = np.maximum(h.reshape(128, 128), 0.0).astype(BF16)
            base = c * NODES_PER_CORE + b * 128
            h2a[base:base + 128] = h.astype(np.float32) @ W2e.astype(BF16).astype(np.float32)

    h2d_bf = h2a[:, :64].astype(BF16)
    als2, ald2 = h2a[:, 64], h2a[:, 65]

    out = np.zeros((N_PAD, OUT_DIM), np.float32)
    for c in range(NC):
        sg, dg = meta["src_g"][c], meta["dst_g"][c]
        v = meta["valid"][c]
        h2g = h2d_bf[sg] * v[:, None]
        z1 = (als2[sg] + ald2[dg]) * v
        z2 = NEG * z1
        ex = np.exp(np.maximum(z1, z2)).astype(np.float32)  # [E]
        hs = (h2g.astype(np.float32) * ex[:, None].astype(BF16).astype(np.float32)).astype(BF16)
        dloc = meta["dloc_g"][c]
        for b in range(N_BLOCKS):
            sl = slice(b * meta["n_bt"] * 128, (b + 1) * meta["n_bt"] * 128)
            oh = (dloc[sl, None] == np.arange(128)[None, :])
            agg = oh.T.astype(np.float32) @ hs[sl].astype(np.float32)
            den = oh.T.astype(np.float32) @ ex[sl, None]
            with np.errstate(divide="ignore", invalid="ignore"):
                o = agg / den
            base = c * NODES_PER_CORE + b * 128
            out[base:base + 128] = o
    res = np.zeros((N_NODES, OUT_DIM), np.float32)
    res[meta["old_of_new"][real]] = out[real]
    return res


# ----------------------------------------------------------------------------
# Bass programs
# ----------------------------------------------------------------------------

def _build_launch_a():
    import concourse.bacc as bacc
    import concourse.mybir as mybir
    import concourse.tile as tile

    nc = bacc.Bacc("TRN2", target_bir_lowering=False, debug=False, num_devices=NC)
    xT = nc.dram_tensor("xT", [128, NODES_PER_CORE], mybir.dt.float32, kind="ExternalInput")
    w1e = nc.dram_tensor("w1e", [128, 144], mybir.dt.float32, kind="ExternalInput")
    featT = nc.dram_tensor("featT", [144, NODES_PER_CORE], mybir.dt.float32, kind="ExternalOutput")
    TS = 256  # 49 * 256 = 12544
    with tile.TileContext(nc) as tc:
        with tc.tile_pool(name="w", bufs=1) as wp, \
             tc.tile_pool(name="s", bufs=6) as sp, \
             tc.tile_pool(name="o", bufs=6) as op, \
             tc.tile_pool(name="ps", bufs=4, space="PSUM") as pp:
            wt = wp.tile([128, 144], mybir.dt.float32)
            nc.sync.dma_start(wt[:], w1e.ap())
            for i in range(NODES_PER_CORE // TS):
                xt = sp.tile([128, TS], mybir.dt.float32, tag="x")
                nc.sync.dma_start(xt[:], xT.ap()[:, i * TS:(i + 1) * TS])
                ps = pp.tile([128, TS], mybir.dt.float32, space="PSUM", tag="ps")
                ps2 = pp.tile([16, TS], mybir.dt.float32, space="PSUM", tag="ps2")
                nc.tensor.matmul(ps[:], wt[:, 0:128], xt[:], start=True, stop=True)
                nc.tensor.matmul(ps2[:], wt[:, 128:144], xt[:], start=True, stop=True)
                ot = op.tile([128, TS], mybir.dt.float32, tag="o")
                ot2 = op.tile([16, TS], mybir.dt.float32, tag="o2")
                nc.vector.tensor_copy(ot[:], ps[:])
                nc.vector.tensor_copy(ot2[:], ps2[:])
                nc.sync.dma_start(featT.ap()[0:128, i * TS:(i + 1) * TS], ot[:])
                nc.sync.dma_start(featT.ap()[128:144, i * TS:(i + 1) * TS], ot2[:])
    nc.compile()
    return nc


def _build_edge_launch(layer, n_bt):
    """layer 1: F=128, heads=4, h2a epilogue; layer 2: F=64, 1 head, out2."""
    import concourse.bacc as bacc
    import concourse.mybir as mybir
    import concourse.tile as tile
    from concourse.masks import make_identity

    F = 128 if layer == 1 else 64
    NH = HEADS if layer == 1 else 1
    CW = F // NH  # channels per head
    ZC = 8 if layer == 1 else 2
    NT = N_BLOCKS * n_bt

    nc = bacc.Bacc("TRN2", target_bir_lowering=False, debug=False, num_devices=NC)
    hdg = nc.dram_tensor("hdg", [N_BLOCKS, 128, n_bt, F], mybir.dt.bfloat16, kind="ExternalInput")
    zg = nc.dram_tensor("zg", [N_BLOCKS, 128, n_bt, ZC], mybir.dt.float32, kind="ExternalInput")
    ohd = nc.dram_tensor("ohd", [N_BLOCKS, 128, n_bt, 128], mybir.dt.uint8, kind="ExternalInput")
    if layer == 1:
        w2e = nc.dram_tensor("w2e", [128, 66], mybir.dt.bfloat16, kind="ExternalInput")
        outt = nc.dram_tensor("h2a", [66, NODES_PER_CORE], mybir.dt.float32, kind="ExternalOutput")
    else:
        outt = nc.dram_tensor("out2", [NODES_PER_CORE, OUT_DIM], mybir.dt.float32, kind="ExternalOutput")

    dt = mybir.dt
    with tile.TileContext(nc) as tc:
        with tc.tile_pool(name="cst", bufs=1) as cp, \
             tc.tile_pool(name="hdgp", bufs=3) as hp, \
             tc.tile_pool(name="zp", bufs=3) as zp, \
             tc.tile_pool(name="zw", bufs=2) as zw, \
             tc.tile_pool(name="exp", bufs=2) as xp, \
             tc.tile_pool(name="hsp", bufs=3) as hsp, \
             tc.tile_pool(name="ohp", bufs=3) as ohp, \
             tc.tile_pool(name="epi", bufs=2) as ep, \
             tc.tile_pool(name="psA", bufs=2, space="PSUM") as psa, \
             tc.tile_pool(name="psB", bufs=2, space="PSUM") as psb, \
             tc.tile_pool(name="psC", bufs=2, space="PSUM") as psc:
            if layer == 1:
                w2t = cp.tile([128, 66], dt.bfloat16)
                nc.sync.dma_start(w2t[:], w2e.ap())
                ident = cp.tile([128, 128], dt.bfloat16)
                make_identity(nc, ident[:])

            for b in range(N_BLOCKS):
                t0 = b * n_bt
                hdg_t = hp.tile([128, n_bt, F], dt.bfloat16, tag="hdg")
                nc.scalar.dma_start(hdg_t[:], hdg.ap()[b])
                zg_t = zp.tile([128, n_bt, ZC], dt.float32, tag="zg")
                nc.sync.dma_start(zg_t[:], zg.ap()[b])
                oh_t = ohp.tile([128, n_bt, 128], dt.bfloat16, tag="oh")
                nc.gpsimd.dma_start(oh_t[:], ohd.ap()[b])

                zm = zw.tile([128, n_bt, NH], dt.float32, tag="zm")
                z2 = zw.tile([128, n_bt, NH], dt.float32, tag="z2")
                nc.vector.tensor_add(zm[:], zg_t[:, :, 0:NH], zg_t[:, :, NH:2 * NH])
                nc.vector.tensor_scalar_mul(z2[:], zm[:], NEG)
                nc.vector.tensor_tensor(out=zm[:], in0=zm[:], in1=z2[:], op=mybir.AluOpType.max)
                # exp with per-head expansion via stride-0 read
                ex = xp.tile([128, n_bt, F], dt.bfloat16, tag="ex")
                zexp = zm[:].unsqueeze(-1).to_broadcast([128, n_bt, NH, CW])
                nc.scalar.activation(ex[:].rearrange("p t (h c) -> p t h c", h=NH), zexp,
                                     mybir.ActivationFunctionType.Exp)
                hs = hsp.tile([128, n_bt, F + NH], dt.bfloat16, tag="hs")
                nc.vector.tensor_mul(hs[:, :, 0:F], hdg_t[:], ex[:])
                nc.vector.tensor_copy(
                    hs[:, :, F:F + NH],
                    ex[:].rearrange("p t (h c) -> p t h c", h=NH)[:, :, :, 0])

                agg = psa.tile([128, F + NH], dt.float32, space="PSUM", tag="agg")
                for t in range(n_bt):
                    nc.tensor.matmul(agg[:], oh_t[:, t, :], hs[:, t, :],
                                     start=(t == 0), stop=(t == n_bt - 1))
                rd = ep.tile([128, NH], dt.float32, tag="rd")
                nc.vector.reciprocal(rd[:], agg[:, F:F + NH])
                if layer == 1:
                    hbf = ep.tile([128, F], dt.bfloat16, tag="hbf")
                    rdx = rd[:].unsqueeze(-1).to_broadcast([128, NH, CW])
                    nc.vector.tensor_tensor(out=hbf[:].rearrange("p (h c) -> p h c", h=NH),
                                            in0=agg[:, 0:F].rearrange("p (h c) -> p h c", h=NH),
                                            in1=rdx, op=mybir.AluOpType.mult)
                    nc.vector.tensor_scalar_max(hbf[:], hbf[:], 0.0)
                    hTp = psc.tile([128, 128], dt.bfloat16, space="PSUM", tag="hT")
                    nc.tensor.transpose(hTp[:], hbf[:], ident[:])
                    hTb = ep.tile([128, 128], dt.bfloat16, tag="hTb")
                    nc.scalar.copy(hTb[:], hTp[:])
                    h2p = psc.tile([66, 128], dt.float32, space="PSUM", tag="h2a")
                    nc.tensor.matmul(h2p[:], w2t[:], hTb[:], start=True, stop=True)
                    h2s = ep.tile([66, 128], dt.float32, tag="h2s")
                    nc.vector.tensor_copy(h2s[:], h2p[:])
                    nc.sync.dma_start(outt.ap()[:, b * 128:(b + 1) * 128], h2s[:])
                else:
                    o2 = ep.tile([128, F], dt.float32, tag="o2")
                    rdx = rd[:].to_broadcast([128, F])
                    nc.vector.tensor_tensor(out=o2[:], in0=agg[:, 0:F], in1=rdx,
                                            op=mybir.AluOpType.mult)
                    nc.sync.dma_start(outt.ap()[b * 128:(b + 1) * 128, :], o2[:])
    nc.compile()
    return nc


# ----------------------------------------------------------------------------
# main entry
# ----------------------------------------------------------------------------

def kernel(x, edge_index, W1, att_src1, att_dst1, b1, W2, att_src2, att_dst2, b2):
    meta = _prep(edge_index)
    W1e = _w1ext(W1, att_src1, att_dst1)
    W2e = _w2ext(W2, att_src2, att_dst2)

    if os.environ.get("GAT_NUMPY"):
        return _run_numpy(x, meta, W1e, W2e)

    from concourse.bass_utils import run_bass_kernel_spmd

    n_bt, NT = meta["n_bt"], meta["NT"]
    old_of_new = meta["old_of_new"]
    real = old_of_new >= 0

    xp = np.zeros((N_PAD, IN_DIM), np.float32)
    xp[real] = np.asarray(x, np.float32)[old_of_new[real]]

    trace = bool(os.environ.get("GAT_TRACE"))
    times = []

    # ---- launch A
    nc_a = _get_cached("A", _build_launch_a)
    in_maps = []
    for c in range(NC):
        sl = slice(c * NODES_PER_CORE, (c + 1) * NODES_PER_CORE)
        in_maps.append({"xT": np.ascontiguousarray(xp[sl].T), "w1e": W1e})
    res = run_bass_kernel_spmd(nc_a, in_maps, core_ids=list(range(NC)), trace=trace)
    times.append(res.exec_time_ns)
    feat = np.concatenate([res.results[c]["featT"].T for c in range(NC)], axis=0)

    hd_bf = feat[:, :128].astype(BF16)
    als, als2x = feat[:, 128:132], feat[:, 132:136]
    ald, ald2x = feat[:, 136:140], feat[:, 140:144]

    eye = np.concatenate([np.eye(128, dtype=np.uint8),
                          np.zeros((1, 128), np.uint8)])

    def _ohot(c):
        dl = meta["dloc_g"][c].astype(np.int64).reshape(N_BLOCKS, meta["n_bt"], 128)
        oh = eye[dl]  # [NB, n_bt, 128p, 128d]
        return np.ascontiguousarray(oh.transpose(0, 2, 1, 3))

    # ---- launch B
    nc_b = _get_cached(("B", n_bt), lambda: _build_edge_launch(1, n_bt))
    in_maps = []
    for c in range(NC):
        sg, dg, v = meta["src_g"][c], meta["dst_g"][c], meta["valid"][c]
        hdgc = hd_bf[sg] * v[:, None]
        z = np.concatenate([als[sg], ald[dg]], axis=1)
        z *= v[:, None]
        in_maps.append({
            "hdg": _pmaj(hdgc, NT), "zg": _pmaj(z.astype(np.float32), NT),
            "ohd": _ohot(c), "w2e": W2e.astype(BF16),
        })
    res = run_bass_kernel_spmd(nc_b, in_maps, core_ids=list(range(NC)), trace=trace)
    times.append(res.exec_time_ns)
    h2a = np.concatenate([res.results[c]["h2a"].T for c in range(NC)], axis=0)

    h2d_bf = h2a[:, :64].astype(BF16)
    als2, ald2 = h2a[:, 64:65], h2a[:, 65:66]

    # ---- launch C
    nc_c = _get_cached(("C", n_bt), lambda: _build_edge_launch(2, n_bt))
    in_maps = []
    for c in range(NC):
        sg, dg, v = meta["src_g"][c], meta["dst_g"][c], meta["valid"][c]
        h2gc = h2d_bf[sg] * v[:, None]
        z = np.concatenate([als2[sg], ald2[dg]], axis=1)
        z *= v[:, None]
        in_maps.append({
            "hdg": _pmaj(h2gc, NT), "zg": _pmaj(z.astype(np.float32), NT),
            "ohd": _ohot(c),
        })
    res = run_bass_kernel_spmd(nc_c, in_maps, core_ids=list(range(NC)), trace=trace)
    times.append(res.exec_time_ns)
    out_pad = np.concatenate([res.results[c]["out2"] for c in range(NC)], axis=0)

    if trace and all(t is not None for t in times):
        kernel.last_exec_ns = sum(times)
        print("per-launch exec ns:", times, "total:", sum(times))

    out = np.zeros((N_NODES, OUT_DIM), np.float32)
    out[old_of_new[real]] = out_pad[real]
    return out


def _get_cached(key, builder):
    if key not in _cache:
        _cache[key] = builder()
    return _cache[key]


# revision 19
# speedup vs baseline: 1.0048x; 1.0048x over previous
"""Trainium2 Bass kernel for a 2-layer GAT (PyG GATConv semantics).

Strategy (8 NeuronCores, SPMD):
  - Host relabels nodes: dsts dealt to 8 cores snake-by-in-degree, grouped
    into 98 blocks of 128 dsts per core (block edge-counts equalized).
  - Edges (incl. self-loops) are dst-sorted per core and padded so every
    block owns exactly n_bt tiles of 128 edge slots -> one uniform SPMD
    program for all cores.
  - Launch A (dense): featT = W1ext.T @ xT per core shard. W1ext packs
    W1 plus per-head attention columns, so als/ald (and 0.2x copies) come
    out of the same matmul, fp32.
  - Host gathers per-edge streams (pure data movement): hd[src] as bf16,
    (als[src], ald[dst], 0.2 als[src], 0.2 ald[dst]) as fp32.
  - Launch B (L1 edge phase): per block: batched z = als+ald, leaky via
    max(z, 0.2z), exp on ACT (expanded per-head), hd_s = hd * ex, onehot
    (is_equal vs iota) per tile, PE matmuls accumulate agg/den in PSUM,
    epilogue normalizes + relu -> h, transposes and applies W2ext to
    produce (h2d | als2 | ald2) per node.
  - Host gathers L2 per-edge streams; Launch C = L2 edge phase -> out2.
All FLOPs happen on device; the host only permutes/gathers/casts.
"""

import os
import numpy as np
import ml_dtypes

N_NODES = 100000
N_EDGES = 1600000
IN_DIM = 128
HID = 128
HEADS = 4
C1 = 32
OUT_DIM = 64
NEG = 0.2
NC = 8
NODES_PER_CORE = 12544  # 98 blocks * 128
N_BLOCKS = 98
REAL_PER_CORE = 12500
N_PAD = NC * NODES_PER_CORE

BF16 = ml_dtypes.bfloat16

_cache = {}


# ----------------------------------------------------------------------------
# Host-side graph preparation (indexing only)
# ----------------------------------------------------------------------------

def _prep(edge_index):
    src0 = np.asarray(edge_index[0], dtype=np.int64)
    dst0 = np.asarray(edge_index[1], dtype=np.int64)
    loop = np.arange(N_NODES, dtype=np.int64)
    src = np.concatenate([src0, loop]).astype(np.int32)
    dst = np.concatenate([dst0, loop]).astype(np.int32)
    E = src.shape[0]

    deg = np.bincount(dst, minlength=N_NODES)
    order = np.argsort(-deg, kind="stable")  # nodes by in-degree desc

    # snake-deal nodes to cores
    i = np.arange(N_NODES)
    r, j = i // NC, i % NC
    core_of_rank = np.where(r % 2 == 0, j, NC - 1 - j)
    # rank within core
    rank_in_core = np.zeros(N_NODES, dtype=np.int64)
    for c in range(NC):
        m = core_of_rank == c
        rank_in_core[m] = np.arange(m.sum())
    # snake-deal a core's nodes into 98 blocks (equalizes block edge sums)
    k = rank_in_core
    rb, jb = k // N_BLOCKS, k % N_BLOCKS
    block_of = np.where(rb % 2 == 0, jb, N_BLOCKS - 1 - jb)
    slot_of = rb  # < 128 since 12500/98 < 128

    new_id = np.empty(N_NODES, dtype=np.int64)
    new_id[order] = core_of_rank * NODES_PER_CORE + block_of * 128 + slot_of
    old_of_new = np.full(N_PAD, -1, dtype=np.int64)
    old_of_new[new_id] = np.arange(N_NODES)

    s_new = new_id[src]
    d_new = new_id[dst]
    core_e = d_new // NODES_PER_CORE

    cores = []
    max_bt = 0
    for c in range(NC):
        m = core_e == c
        sc, dc = s_new[m], d_new[m]
        o = np.argsort(dc, kind="stable")
        sc, dc = sc[o], dc[o]
        dloc = dc - c * NODES_PER_CORE
        blk = dloc // 128
        cnt = np.bincount(blk, minlength=N_BLOCKS)
        max_bt = max(max_bt, int(np.ceil(cnt.max() / 128)))
        cores.append((sc, dloc, blk, cnt))

    n_bt = max_bt
    NT = N_BLOCKS * n_bt
    E_pad = NT * 128

    src_g = np.zeros((NC, E_pad), dtype=np.int64)   # new-node id of edge src
    dst_g = np.zeros((NC, E_pad), dtype=np.int64)   # new-node id of edge dst
    dloc_g = np.full((NC, E_pad), 128.0, dtype=np.float32)  # sentinel 128
    valid = np.zeros((NC, E_pad), dtype=bool)
    for c in range(NC):
        sc, dloc, blk, cnt = cores[c]
        ofs = 0
        pos = np.empty(len(sc), dtype=np.int64)
        start = np.concatenate([[0], np.cumsum(cnt)[:-1]])
        for b in range(N_BLOCKS):
            sl = slice(start[b], start[b] + cnt[b])
            pos[sl] = b * n_bt * 128 + np.arange(cnt[b])
        src_g[c, pos] = sc
        dst_g[c, pos] = dloc + c * NODES_PER_CORE
        dloc_g[c, pos] = (dloc % 128).astype(np.float32)
        valid[c, pos] = True

    return dict(n_bt=n_bt, NT=NT, E_pad=E_pad, old_of_new=old_of_new,
                new_id=new_id, src_g=src_g, dst_g=dst_g, dloc_g=dloc_g,
                valid=valid)


def _w1ext(W1, att_src1, att_dst1):
    # [128, 144] fp32: W1 | asrc blockdiag | 0.2 asrc | adst | 0.2 adst
    W1 = np.asarray(W1, np.float32)
    a_s = np.asarray(att_src1, np.float32)
    a_d = np.asarray(att_dst1, np.float32)
    bs = np.zeros((IN_DIM, HEADS), np.float32)
    bd = np.zeros((IN_DIM, HEADS), np.float32)
    # als[n,h] = sum_c hd[n, 32h+c]*a_s[h,c] = x @ (W1 @ asrc_bd)
    asrc_bd = np.zeros((HID, HEADS), np.float32)
    adst_bd = np.zeros((HID, HEADS), np.float32)
    for h in range(HEADS):
        asrc_bd[32 * h:32 * h + 32, h] = a_s[h]
        adst_bd[32 * h:32 * h + 32, h] = a_d[h]
    ws = W1 @ asrc_bd
    wd = W1 @ adst_bd
    return np.concatenate([W1, ws, NEG * ws, wd, NEG * wd], axis=1)


def _w2ext(W2, att_src2, att_dst2):
    W2 = np.asarray(W2, np.float32)
    a2s = np.asarray(att_src2, np.float32).reshape(-1)
    a2d = np.asarray(att_dst2, np.float32).reshape(-1)
    ws = (W2 @ a2s)[:, None]
    wd = (W2 @ a2d)[:, None]
    return np.concatenate([W2, ws, wd], axis=1)  # [128, 66]


def _pmaj(arr, NT):
    # [E_pad, F] -> [N_BLOCKS, 128, n_bt, F]; edge (b, t, p) at [b, p, t]
    F = arr.shape[1] if arr.ndim == 2 else 1
    n_bt = NT // N_BLOCKS
    a = arr.reshape(N_BLOCKS, n_bt, 128, F)
    return np.ascontiguousarray(a.transpose(0, 2, 1, 3))


# ----------------------------------------------------------------------------
# numpy emulation of the device dataflow (for validation)
# ----------------------------------------------------------------------------

def _run_numpy(x, meta, W1e, W2e):
    n_bt, NT = meta["n_bt"], meta["NT"]
    xp = np.zeros((N_PAD, IN_DIM), np.float32)
    real = meta["old_of_new"] >= 0
    xp[real] = np.asarray(x, np.float32)[meta["old_of_new"][real]]

    # Launch A: featT per core
    feat = xp @ W1e  # [N_PAD, 144]
    hd_bf = feat[:, :128].astype(BF16)
    als, als2x = feat[:, 128:132], feat[:, 132:136]
    ald, ald2x = feat[:, 136:140], feat[:, 140:144]

    h2a = np.zeros((N_PAD, 66), np.float32)
    for c in range(NC):
        sg, dg = meta["src_g"][c], meta["dst_g"][c]
        v = meta["valid"][c]
        hdg = hd_bf[sg] * v[:, None]
        z1 = (als[sg] + ald[dg]) * v[:, None]
        z2 = (als2x[sg] + ald2x[dg]) * v[:, None]
        ex = np.exp(np.maximum(z1, z2)).astype(np.float32)  # [E,4]
        exx = np.repeat(ex, 32, axis=1).astype(BF16)
        hs = (hdg.astype(np.float32) * exx.astype(np.float32)).astype(BF16)
        dloc = meta["dloc_g"][c]
        for b in range(N_BLOCKS):
            sl = slice(b * n_bt * 128, (b + 1) * n_bt * 128)
            oh = (dloc[sl, None] == np.arange(128)[None, :])  # [Eb, 128]
            agg = oh.T.astype(np.float32) @ hs[sl].astype(np.float32)
            den = oh.T.astype(np.float32) @ ex[sl]
            with np.errstate(divide="ignore", invalid="ignore"):
                rden = 1.0 / den
            h = agg.reshape(128, 4, 32) * rden[:, :, None]
            h = np.maximum(h.reshape(128, 128), 0.0).astype(BF16)
            base = c * NODES_PER_CORE + b * 128
            h2a[base:base + 128] = h.astype(np.float32) @ W2e.astype(BF16).astype(np.float32)

    h2d_bf = h2a[:, :64].astype(BF16)
    als2, ald2 = h2a[:, 64], h2a[:, 65]

    out = np.zeros((N_PAD, OUT_DIM), np.float32)
    for c in range(NC):
        sg, dg = meta["src_g"][c], meta["dst_g"][c]
        v = meta["valid"][c]
        h2g = h2d_bf[sg] * v[:, None]
        z1 = (als2[sg] + ald2[dg]) * v
        z2 = NEG * z1
        ex = np.exp(np.maximum(z1, z2)).astype(np.float32)  # [E]
        hs = (h2g.astype(np.float32) * ex[:, None].astype(BF16).astype(np.float32)).astype(BF16)
        dloc = meta["dloc_g"][c]
        for b in range(N_BLOCKS):
            sl = slice(b * meta["n_bt"] * 128, (b + 1) * meta["n_bt"] * 128)
            oh = (dloc[sl, None] == np.arange(128)[None, :])
            agg = oh.T.astype(np.float32) @ hs[sl].astype(np.float32)
            den = oh.T.astype(np.float32) @ ex[sl, None]
            with np.errstate(divide="ignore", invalid="ignore"):
                o = agg / den
            base = c * NODES_PER_CORE + b * 128
            out[base:base + 128] = o
    res = np.zeros((N_NODES, OUT_DIM), np.float32)
    res[meta["old_of_new"][real]] = out[real]
    return res


# ----------------------------------------------------------------------------
# Bass programs
# ----------------------------------------------------------------------------

def _build_launch_a():
    import concourse.bacc as bacc
    import concourse.mybir as mybir
    import concourse.tile as tile

    nc = bacc.Bacc("TRN2", target_bir_lowering=False, debug=False, num_devices=NC)
    xT = nc.dram_tensor("xT", [128, NODES_PER_CORE], mybir.dt.float32, kind="ExternalInput")
    w1e = nc.dram_tensor("w1e", [128, 144], mybir.dt.float32, kind="ExternalInput")
    featT = nc.dram_tensor("featT", [144, NODES_PER_CORE], mybir.dt.float32, kind="ExternalOutput")
    TS = 256  # 49 * 256 = 12544
    with tile.TileContext(nc) as tc:
        with tc.tile_pool(name="w", bufs=1) as wp, \
             tc.tile_pool(name="s", bufs=6) as sp, \
             tc.tile_pool(name="o", bufs=6) as op, \
             tc.tile_pool(name="ps", bufs=4, space="PSUM") as pp:
            wt = wp.tile([128, 144], mybir.dt.float32)
            nc.sync.dma_start(wt[:], w1e.ap())
            for i in range(NODES_PER_CORE // TS):
                xt = sp.tile([128, TS], mybir.dt.float32, tag="x")
                nc.sync.dma_start(xt[:], xT.ap()[:, i * TS:(i + 1) * TS])
                ps = pp.tile([128, TS], mybir.dt.float32, space="PSUM", tag="ps")
                ps2 = pp.tile([16, TS], mybir.dt.float32, space="PSUM", tag="ps2")
                nc.tensor.matmul(ps[:], wt[:, 0:128], xt[:], start=True, stop=True)
                nc.tensor.matmul(ps2[:], wt[:, 128:144], xt[:], start=True, stop=True)
                ot = op.tile([128, TS], mybir.dt.float32, tag="o")
                ot2 = op.tile([16, TS], mybir.dt.float32, tag="o2")
                nc.vector.tensor_copy(ot[:], ps[:])
                nc.vector.tensor_copy(ot2[:], ps2[:])
                nc.sync.dma_start(featT.ap()[0:128, i * TS:(i + 1) * TS], ot[:])
                nc.sync.dma_start(featT.ap()[128:144, i * TS:(i + 1) * TS], ot2[:])
    nc.compile()
    return nc


def _build_edge_launch(layer, n_bt):
    """layer 1: F=128, heads=4, h2a epilogue; layer 2: F=64, 1 head, out2."""
    import concourse.bacc as bacc
    import concourse.mybir as mybir
    import concourse.tile as tile
    from concourse.masks import make_identity

    F = 128 if layer == 1 else 64
    NH = HEADS if layer == 1 else 1
    CW = F // NH  # channels per head
    ZC = 8 if layer == 1 else 2
    NT = N_BLOCKS * n_bt

    nc = bacc.Bacc("TRN2", target_bir_lowering=False, debug=False, num_devices=NC)
    hdg = nc.dram_tensor("hdg", [N_BLOCKS, 128, n_bt, F], mybir.dt.bfloat16, kind="ExternalInput")
    zg = nc.dram_tensor("zg", [N_BLOCKS, 128, n_bt, ZC], mybir.dt.float32, kind="ExternalInput")
    ohd = nc.dram_tensor("ohd", [N_BLOCKS, 128, n_bt, 128], mybir.dt.uint8, kind="ExternalInput")
    if layer == 1:
        w2e = nc.dram_tensor("w2e", [128, 66], mybir.dt.bfloat16, kind="ExternalInput")
        outt = nc.dram_tensor("h2a", [66, NODES_PER_CORE], mybir.dt.float32, kind="ExternalOutput")
    else:
        outt = nc.dram_tensor("out2", [NODES_PER_CORE, OUT_DIM], mybir.dt.float32, kind="ExternalOutput")

    dt = mybir.dt
    with tile.TileContext(nc) as tc:
        with tc.tile_pool(name="cst", bufs=1) as cp, \
             tc.tile_pool(name="hdgp", bufs=3) as hp, \
             tc.tile_pool(name="zp", bufs=3) as zp, \
             tc.tile_pool(name="zw", bufs=2) as zw, \
             tc.tile_pool(name="exp", bufs=2) as xp, \
             tc.tile_pool(name="hsp", bufs=3) as hsp, \
             tc.tile_pool(name="ohp", bufs=3) as ohp, \
             tc.tile_pool(name="epi", bufs=2) as ep, \
             tc.tile_pool(name="psA", bufs=2, space="PSUM") as psa, \
             tc.tile_pool(name="psB", bufs=2, space="PSUM") as psb, \
             tc.tile_pool(name="psC", bufs=2, space="PSUM") as psc:
            if layer == 1:
                w2t = cp.tile([128, 66], dt.bfloat16)
                nc.sync.dma_start(w2t[:], w2e.ap())
                ident = cp.tile([128, 128], dt.bfloat16)
                make_identity(nc, ident[:])

            for b in range(N_BLOCKS):
                t0 = b * n_bt
                hdg_t = hp.tile([128, n_bt, F], dt.bfloat16, tag="hdg")
                nc.scalar.dma_start(hdg_t[:], hdg.ap()[b])
                zg_t = zp.tile([128, n_bt, ZC], dt.float32, tag="zg")
                nc.sync.dma_start(zg_t[:], zg.ap()[b])
                oh_t = ohp.tile([128, n_bt, 128], dt.bfloat16, tag="oh")
                nc.gpsimd.dma_start(oh_t[:], ohd.ap()[b])

                zm = zw.tile([128, n_bt, NH], dt.float32, tag="zm")
                z2 = zw.tile([128, n_bt, NH], dt.float32, tag="z2")
                nc.vector.tensor_add(zm[:], zg_t[:, :, 0:NH], zg_t[:, :, NH:2 * NH])
                nc.vector.tensor_scalar_mul(z2[:], zm[:], NEG)
                nc.vector.tensor_tensor(out=zm[:], in0=zm[:], in1=z2[:], op=mybir.AluOpType.max)
                # exp with per-head expansion via stride-0 read
                ex = xp.tile([128, n_bt, F], dt.bfloat16, tag="ex")
                zexp = zm[:].unsqueeze(-1).to_broadcast([128, n_bt, NH, CW])
                nc.scalar.activation(ex[:].rearrange("p t (h c) -> p t h c", h=NH), zexp,
                                     mybir.ActivationFunctionType.Exp)
                FW = F + NH if layer == 2 else F
                hs = hsp.tile([128, n_bt, FW], dt.bfloat16, tag="hs")
                nc.vector.tensor_mul(hs[:, :, 0:F], hdg_t[:], ex[:])
                if layer == 2:
                    nc.vector.tensor_copy(
                        hs[:, :, F:F + NH],
                        ex[:].rearrange("p t (h c) -> p t h c", h=NH)[:, :, :, 0])

                agg = psa.tile([128, FW], dt.float32, space="PSUM", tag="agg")
                den = None
                if layer == 1:
                    den = psb.tile([128, NH], dt.float32, space="PSUM", tag="den")
                for t in range(n_bt):
                    nc.tensor.matmul(agg[:], oh_t[:, t, :], hs[:, t, :],
                                     start=(t == 0), stop=(t == n_bt - 1))
                    if layer == 1:
                        exs = ex[:].rearrange("p t (h c) -> p t h c", h=NH)[:, t, :, 0]
                        nc.tensor.matmul(den[:], oh_t[:, t, :], exs,
                                         start=(t == 0), stop=(t == n_bt - 1))
                rd = ep.tile([128, NH], dt.float32, tag="rd")
                nc.vector.reciprocal(rd[:], den[:] if layer == 1 else agg[:, F:F + NH])
                if layer == 1:
                    hbf = ep.tile([128, F], dt.bfloat16, tag="hbf")
                    rdx = rd[:].unsqueeze(-1).to_broadcast([128, NH, CW])
                    nc.vector.tensor_tensor(out=hbf[:].rearrange("p (h c) -> p h c", h=NH),
                                            in0=agg[:, 0:F].rearrange("p (h c) -> p h c", h=NH),
                                            in1=rdx, op=mybir.AluOpType.mult)
                    nc.vector.tensor_scalar_max(hbf[:], hbf[:], 0.0)
                    hTp = psc.tile([128, 128], dt.bfloat16, space="PSUM", tag="hT")
                    nc.tensor.transpose(hTp[:], hbf[:], ident[:])
                    hTb = ep.tile([128, 128], dt.bfloat16, tag="hTb")
                    nc.scalar.copy(hTb[:], hTp[:])
                    h2p = psc.tile([66, 128], dt.float32, space="PSUM", tag="h2a")
                    nc.tensor.matmul(h2p[:], w2t[:], hTb[:], start=True, stop=True)
                    h2s = ep.tile([66, 128], dt.float32, tag="h2s")
                    nc.vector.tensor_copy(h2s[:], h2p[:])
                    nc.sync.dma_start(outt.ap()[:, b * 128:(b + 1) * 128], h2s[:])
                else:
                    o2 = ep.tile([128, F], dt.float32, tag="o2")
                    rdx = rd[:].to_broadcast([128, F])
                    nc.vector.tensor_tensor(out=o2[:], in0=agg[:, 0:F], in1=rdx,
                                            op=mybir.AluOpType.mult)
                    nc.sync.dma_start(outt.ap()[b * 128:(b + 1) * 128, :], o2[:])
    nc.compile()
    return nc


# ----------------------------------------------------------------------------
# main entry
# ----------------------------------------------------------------------------

def kernel(x, edge_index, W1, att_src1, att_dst1, b1, W2, att_src2, att_dst2, b2):
    meta = _prep(edge_index)
    W1e = _w1ext(W1, att_src1, att_dst1)
    W2e = _w2ext(W2, att_src2, att_dst2)

    if os.environ.get("GAT_NUMPY"):
        return _run_numpy(x, meta, W1e, W2e)

    from concourse.bass_utils import run_bass_kernel_spmd

    n_bt, NT = meta["n_bt"], meta["NT"]
    old_of_new = meta["old_of_new"]
    real = old_of_new >= 0

    xp = np.zeros((N_PAD, IN_DIM), np.float32)
    xp[real] = np.asarray(x, np.float32)[old_of_new[real]]

    trace = bool(os.environ.get("GAT_TRACE"))
    times = []

    # ---- launch A
    nc_a = _get_cached("A", _build_launch_a)
    in_maps = []
    for c in range(NC):
        sl = slice(c * NODES_PER_CORE, (c + 1) * NODES_PER_CORE)
        in_maps.append({"xT": np.ascontiguousarray(xp[sl].T), "w1e": W1e})
    res = run_bass_kernel_spmd(nc_a, in_maps, core_ids=list(range(NC)), trace=trace)
    times.append(res.exec_time_ns)
    feat = np.concatenate([res.results[c]["featT"].T for c in range(NC)], axis=0)

    hd_bf = feat[:, :128].astype(BF16)
    als, als2x = feat[:, 128:132], feat[:, 132:136]
    ald, ald2x = feat[:, 136:140], feat[:, 140:144]

    eye = np.concatenate([np.eye(128, dtype=np.uint8),
                          np.zeros((1, 128), np.uint8)])

    def _ohot(c):
        dl = meta["dloc_g"][c].astype(np.int64).reshape(N_BLOCKS, meta["n_bt"], 128)
        oh = eye[dl]  # [NB, n_bt, 128p, 128d]
        return np.ascontiguousarray(oh.transpose(0, 2, 1, 3))

    # ---- launch B
    nc_b = _get_cached(("B", n_bt), lambda: _build_edge_launch(1, n_bt))
    in_maps = []
    for c in range(NC):
        sg, dg, v = meta["src_g"][c], meta["dst_g"][c], meta["valid"][c]
        hdgc = hd_bf[sg] * v[:, None]
        z = np.concatenate([als[sg], ald[dg]], axis=1)
        z *= v[:, None]
        in_maps.append({
            "hdg": _pmaj(hdgc, NT), "zg": _pmaj(z.astype(np.float32), NT),
            "ohd": _ohot(c), "w2e": W2e.astype(BF16),
        })
    res = run_bass_kernel_spmd(nc_b, in_maps, core_ids=list(range(NC)), trace=trace)
    times.append(res.exec_time_ns)
    h2a = np.concatenate([res.results[c]["h2a"].T for c in range(NC)], axis=0)

    h2d_bf = h2a[:, :64].astype(BF16)
    als2, ald2 = h2a[:, 64:65], h2a[:, 65:66]

    # ---- launch C
    nc_c = _get_cached(("C", n_bt), lambda: _build_edge_launch(2, n_bt))
    in_maps = []
    for c in range(NC):
        sg, dg, v = meta["src_g"][c], meta["dst_g"][c], meta["valid"][c]
        h2gc = h2d_bf[sg] * v[:, None]
        z = np.concatenate([als2[sg], ald2[dg]], axis=1)
        z *= v[:, None]
        in_maps.append({
            "hdg": _pmaj(h2gc, NT), "zg": _pmaj(z.astype(np.float32), NT),
            "ohd": _ohot(c),
        })
    res = run_bass_kernel_spmd(nc_c, in_maps, core_ids=list(range(NC)), trace=trace)
    times.append(res.exec_time_ns)
    out_pad = np.concatenate([res.results[c]["out2"] for c in range(NC)], axis=0)

    if trace and all(t is not None for t in times):
        kernel.last_exec_ns = sum(times)
        print("per-launch exec ns:", times, "total:", sum(times))

    out = np.zeros((N_NODES, OUT_DIM), np.float32)
    out[old_of_new[real]] = out_pad[real]
    return out


def _get_cached(key, builder):
    if key not in _cache:
        _cache[key] = builder()
    return _cache[key]


# revision 20
# speedup vs baseline: 1.0450x; 1.0400x over previous
"""Trainium2 Bass kernel for a 2-layer GAT (PyG GATConv semantics).

Strategy (8 NeuronCores, SPMD):
  - Host relabels nodes: dsts dealt to 8 cores snake-by-in-degree, grouped
    into 98 blocks of 128 dsts per core (block edge-counts equalized).
  - Edges (incl. self-loops) are dst-sorted per core and padded so every
    block owns exactly n_bt tiles of 128 edge slots -> one uniform SPMD
    program for all cores.
  - Launch A (dense): featT = W1ext.T @ xT per core shard. W1ext packs
    W1 plus per-head attention columns, so als/ald (and 0.2x copies) come
    out of the same matmul, fp32.
  - Host gathers per-edge streams (pure data movement): hd[src] as bf16,
    (als[src], ald[dst], 0.2 als[src], 0.2 ald[dst]) as fp32.
  - Launch B (L1 edge phase): per block: batched z = als+ald, leaky via
    max(z, 0.2z), exp on ACT (expanded per-head), hd_s = hd * ex, onehot
    (is_equal vs iota) per tile, PE matmuls accumulate agg/den in PSUM,
    epilogue normalizes + relu -> h, transposes and applies W2ext to
    produce (h2d | als2 | ald2) per node.
  - Host gathers L2 per-edge streams; Launch C = L2 edge phase -> out2.
All FLOPs happen on device; the host only permutes/gathers/casts.
"""

import os
import numpy as np
import ml_dtypes

N_NODES = 100000
N_EDGES = 1600000
IN_DIM = 128
HID = 128
HEADS = 4
C1 = 32
OUT_DIM = 64
NEG = 0.2
NC = 8
NODES_PER_CORE = 12544  # 98 blocks * 128
N_BLOCKS = 98
REAL_PER_CORE = 12500
N_PAD = NC * NODES_PER_CORE

BF16 = ml_dtypes.bfloat16

_cache = {}


# ----------------------------------------------------------------------------
# Host-side graph preparation (indexing only)
# ----------------------------------------------------------------------------

def _prep(edge_index):
    src0 = np.asarray(edge_index[0], dtype=np.int64)
    dst0 = np.asarray(edge_index[1], dtype=np.int64)
    loop = np.arange(N_NODES, dtype=np.int64)
    src = np.concatenate([src0, loop]).astype(np.int32)
    dst = np.concatenate([dst0, loop]).astype(np.int32)
    E = src.shape[0]

    deg = np.bincount(dst, minlength=N_NODES)
    order = np.argsort(-deg, kind="stable")  # nodes by in-degree desc

    # snake-deal nodes to cores
    i = np.arange(N_NODES)
    r, j = i // NC, i % NC
    core_of_rank = np.where(r % 2 == 0, j, NC - 1 - j)
    # rank within core
    rank_in_core = np.zeros(N_NODES, dtype=np.int64)
    for c in range(NC):
        m = core_of_rank == c
        rank_in_core[m] = np.arange(m.sum())
    # snake-deal a core's nodes into 98 blocks (equalizes block edge sums)
    k = rank_in_core
    rb, jb = k // N_BLOCKS, k % N_BLOCKS
    block_of = np.where(rb % 2 == 0, jb, N_BLOCKS - 1 - jb)
    slot_of = rb  # < 128 since 12500/98 < 128

    new_id = np.empty(N_NODES, dtype=np.int64)
    new_id[order] = core_of_rank * NODES_PER_CORE + block_of * 128 + slot_of
    old_of_new = np.full(N_PAD, -1, dtype=np.int64)
    old_of_new[new_id] = np.arange(N_NODES)

    s_new = new_id[src]
    d_new = new_id[dst]
    core_e = d_new // NODES_PER_CORE

    cores = []
    max_bt = 0
    for c in range(NC):
        m = core_e == c
        sc, dc = s_new[m], d_new[m]
        o = np.argsort(dc, kind="stable")
        sc, dc = sc[o], dc[o]
        dloc = dc - c * NODES_PER_CORE
        blk = dloc // 128
        cnt = np.bincount(blk, minlength=N_BLOCKS)
        max_bt = max(max_bt, int(np.ceil(cnt.max() / 128)))
        cores.append((sc, dloc, blk, cnt))

    n_bt = max_bt
    NT = N_BLOCKS * n_bt
    E_pad = NT * 128

    src_g = np.zeros((NC, E_pad), dtype=np.int64)   # new-node id of edge src
    dst_g = np.zeros((NC, E_pad), dtype=np.int64)   # new-node id of edge dst
    dloc_g = np.full((NC, E_pad), 128.0, dtype=np.float32)  # sentinel 128
    valid = np.zeros((NC, E_pad), dtype=bool)
    for c in range(NC):
        sc, dloc, blk, cnt = cores[c]
        ofs = 0
        pos = np.empty(len(sc), dtype=np.int64)
        start = np.concatenate([[0], np.cumsum(cnt)[:-1]])
        for b in range(N_BLOCKS):
            sl = slice(start[b], start[b] + cnt[b])
            pos[sl] = b * n_bt * 128 + np.arange(cnt[b])
        src_g[c, pos] = sc
        dst_g[c, pos] = dloc + c * NODES_PER_CORE
        dloc_g[c, pos] = (dloc % 128).astype(np.float32)
        valid[c, pos] = True

    return dict(n_bt=n_bt, NT=NT, E_pad=E_pad, old_of_new=old_of_new,
                new_id=new_id, src_g=src_g, dst_g=dst_g, dloc_g=dloc_g,
                valid=valid)


def _w1ext(W1, att_src1, att_dst1):
    # [128, 144] fp32: W1 | asrc blockdiag | 0.2 asrc | adst | 0.2 adst
    W1 = np.asarray(W1, np.float32)
    a_s = np.asarray(att_src1, np.float32)
    a_d = np.asarray(att_dst1, np.float32)
    bs = np.zeros((IN_DIM, HEADS), np.float32)
    bd = np.zeros((IN_DIM, HEADS), np.float32)
    # als[n,h] = sum_c hd[n, 32h+c]*a_s[h,c] = x @ (W1 @ asrc_bd)
    asrc_bd = np.zeros((HID, HEADS), np.float32)
    adst_bd = np.zeros((HID, HEADS), np.float32)
    for h in range(HEADS):
        asrc_bd[32 * h:32 * h + 32, h] = a_s[h]
        adst_bd[32 * h:32 * h + 32, h] = a_d[h]
    ws = W1 @ asrc_bd
    wd = W1 @ adst_bd
    return np.concatenate([W1, ws, NEG * ws, wd, NEG * wd], axis=1)


def _w2ext(W2, att_src2, att_dst2):
    W2 = np.asarray(W2, np.float32)
    a2s = np.asarray(att_src2, np.float32).reshape(-1)
    a2d = np.asarray(att_dst2, np.float32).reshape(-1)
    ws = (W2 @ a2s)[:, None]
    wd = (W2 @ a2d)[:, None]
    return np.concatenate([W2, ws, wd], axis=1)  # [128, 66]


def _pmaj(arr, NT):
    # [E_pad, F] -> [N_BLOCKS, 128, n_bt, F]; edge (b, t, p) at [b, p, t]
    F = arr.shape[1] if arr.ndim == 2 else 1
    n_bt = NT // N_BLOCKS
    a = arr.reshape(N_BLOCKS, n_bt, 128, F)
    return np.ascontiguousarray(a.transpose(0, 2, 1, 3))


# ----------------------------------------------------------------------------
# numpy emulation of the device dataflow (for validation)
# ----------------------------------------------------------------------------

def _run_numpy(x, meta, W1e, W2e):
    n_bt, NT = meta["n_bt"], meta["NT"]
    xp = np.zeros((N_PAD, IN_DIM), np.float32)
    real = meta["old_of_new"] >= 0
    xp[real] = np.asarray(x, np.float32)[meta["old_of_new"][real]]

    # Launch A: featT per core
    feat = xp @ W1e  # [N_PAD, 144]
    hd_bf = feat[:, :128].astype(BF16)
    als, als2x = feat[:, 128:132], feat[:, 132:136]
    ald, ald2x = feat[:, 136:140], feat[:, 140:144]

    h2a = np.zeros((N_PAD, 66), np.float32)
    for c in range(NC):
        sg, dg = meta["src_g"][c], meta["dst_g"][c]
        v = meta["valid"][c]
        hdg = hd_bf[sg] * v[:, None]
        z1 = (als[sg] + ald[dg]) * v[:, None]
        z2 = (als2x[sg] + ald2x[dg]) * v[:, None]
        ex = np.exp(np.maximum(z1, z2)).astype(np.float32)  # [E,4]
        exx = np.repeat(ex, 32, axis=1).astype(BF16)
        hs = (hdg.astype(np.float32) * exx.astype(np.float32)).astype(BF16)
        dloc = meta["dloc_g"][c]
        for b in range(N_BLOCKS):
            sl = slice(b * n_bt * 128, (b + 1) * n_bt * 128)
            oh = (dloc[sl, None] == np.arange(128)[None, :])  # [Eb, 128]
            agg = oh.T.astype(np.float32) @ hs[sl].astype(np.float32)
            den = oh.T.astype(np.float32) @ ex[sl]
            with np.errstate(divide="ignore", invalid="ignore"):
                rden = 1.0 / den
            h = agg.reshape(128, 4, 32) * rden[:, :, None]
            h = np.maximum(h.reshape(128, 128), 0.0).astype(BF16)
            base = c * NODES_PER_CORE + b * 128
            h2a[base:base + 128] = h.astype(np.float32) @ W2e.astype(BF16).astype(np.float32)

    h2d_bf = h2a[:, :64].astype(BF16)
    als2, ald2 = h2a[:, 64], h2a[:, 65]

    out = np.zeros((N_PAD, OUT_DIM), np.float32)
    for c in range(NC):
        sg, dg = meta["src_g"][c], meta["dst_g"][c]
        v = meta["valid"][c]
        h2g = h2d_bf[sg] * v[:, None]
        z1 = (als2[sg] + ald2[dg]) * v
        z2 = NEG * z1
        ex = np.exp(np.maximum(z1, z2)).astype(np.float32)  # [E]
        hs = (h2g.astype(np.float32) * ex[:, None].astype(BF16).astype(np.float32)).astype(BF16)
        dloc = meta["dloc_g"][c]
        for b in range(N_BLOCKS):
            sl = slice(b * meta["n_bt"] * 128, (b + 1) * meta["n_bt"] * 128)
            oh = (dloc[sl, None] == np.arange(128)[None, :])
            agg = oh.T.astype(np.float32) @ hs[sl].astype(np.float32)
            den = oh.T.astype(np.float32) @ ex[sl, None]
            with np.errstate(divide="ignore", invalid="ignore"):
                o = agg / den
            base = c * NODES_PER_CORE + b * 128
            out[base:base + 128] = o
    res = np.zeros((N_NODES, OUT_DIM), np.float32)
    res[meta["old_of_new"][real]] = out[real]
    return res


# ----------------------------------------------------------------------------
# Bass programs
# ----------------------------------------------------------------------------

def _build_launch_a():
    import concourse.bacc as bacc
    import concourse.mybir as mybir
    import concourse.tile as tile

    nc = bacc.Bacc("TRN2", target_bir_lowering=False, debug=False, num_devices=NC)
    xT = nc.dram_tensor("xT", [128, NODES_PER_CORE], mybir.dt.float32, kind="ExternalInput")
    w1e = nc.dram_tensor("w1e", [128, 144], mybir.dt.float32, kind="ExternalInput")
    featT = nc.dram_tensor("featT", [144, NODES_PER_CORE], mybir.dt.float32, kind="ExternalOutput")
    TS = 256  # 49 * 256 = 12544
    with tile.TileContext(nc) as tc:
        with tc.tile_pool(name="w", bufs=1) as wp, \
             tc.tile_pool(name="s", bufs=6) as sp, \
             tc.tile_pool(name="o", bufs=6) as op, \
             tc.tile_pool(name="ps", bufs=4, space="PSUM") as pp:
            wt = wp.tile([128, 144], mybir.dt.float32)
            nc.sync.dma_start(wt[:], w1e.ap())
            for i in range(NODES_PER_CORE // TS):
                xt = sp.tile([128, TS], mybir.dt.float32, tag="x")
                nc.sync.dma_start(xt[:], xT.ap()[:, i * TS:(i + 1) * TS])
                ps = pp.tile([128, TS], mybir.dt.float32, space="PSUM", tag="ps")
                ps2 = pp.tile([16, TS], mybir.dt.float32, space="PSUM", tag="ps2")
                nc.tensor.matmul(ps[:], wt[:, 0:128], xt[:], start=True, stop=True)
                nc.tensor.matmul(ps2[:], wt[:, 128:144], xt[:], start=True, stop=True)
                ot = op.tile([128, TS], mybir.dt.float32, tag="o")
                ot2 = op.tile([16, TS], mybir.dt.float32, tag="o2")
                nc.vector.tensor_copy(ot[:], ps[:])
                nc.vector.tensor_copy(ot2[:], ps2[:])
                nc.sync.dma_start(featT.ap()[0:128, i * TS:(i + 1) * TS], ot[:])
                nc.sync.dma_start(featT.ap()[128:144, i * TS:(i + 1) * TS], ot2[:])
    nc.compile()
    return nc


def _build_edge_launch(layer, n_bt):
    """layer 1: F=128, heads=4, h2a epilogue; layer 2: F=64, 1 head, out2."""
    import concourse.bacc as bacc
    import concourse.mybir as mybir
    import concourse.tile as tile
    from concourse.masks import make_identity

    F = 128 if layer == 1 else 64
    NH = HEADS if layer == 1 else 1
    CW = F // NH  # channels per head
    ZC = 8 if layer == 1 else 2
    NT = N_BLOCKS * n_bt

    nc = bacc.Bacc("TRN2", target_bir_lowering=False, debug=False, num_devices=NC)
    hdg = nc.dram_tensor("hdg", [N_BLOCKS, 128, n_bt, F], mybir.dt.bfloat16, kind="ExternalInput")
    zg = nc.dram_tensor("zg", [N_BLOCKS, 128, n_bt, ZC], mybir.dt.float32, kind="ExternalInput")
    ohd = nc.dram_tensor("ohd", [N_BLOCKS, 128, n_bt, 128], mybir.dt.uint8, kind="ExternalInput")
    if layer == 1:
        w2e = nc.dram_tensor("w2e", [128, 66], mybir.dt.bfloat16, kind="ExternalInput")
        outt = nc.dram_tensor("h2a", [66, NODES_PER_CORE], mybir.dt.float32, kind="ExternalOutput")
    else:
        outt = nc.dram_tensor("out2", [NODES_PER_CORE, OUT_DIM], mybir.dt.float32, kind="ExternalOutput")

    dt = mybir.dt
    with tile.TileContext(nc) as tc:
        with tc.tile_pool(name="cst", bufs=1) as cp, \
             tc.tile_pool(name="hdgp", bufs=4) as hp, \
             tc.tile_pool(name="zp", bufs=4) as zp, \
             tc.tile_pool(name="zw", bufs=3) as zw, \
             tc.tile_pool(name="exp", bufs=3) as xp, \
             tc.tile_pool(name="hsp", bufs=4) as hsp, \
             tc.tile_pool(name="ohp", bufs=4) as ohp, \
             tc.tile_pool(name="epi", bufs=3) as ep, \
             tc.tile_pool(name="psA", bufs=2, space="PSUM") as psa, \
             tc.tile_pool(name="psB", bufs=2, space="PSUM") as psb, \
             tc.tile_pool(name="psC", bufs=2, space="PSUM") as psc:
            if layer == 1:
                w2t = cp.tile([128, 66], dt.bfloat16)
                nc.sync.dma_start(w2t[:], w2e.ap())
                ident = cp.tile([128, 128], dt.bfloat16)
                make_identity(nc, ident[:])

            for b in range(N_BLOCKS):
                t0 = b * n_bt
                hdg_t = hp.tile([128, n_bt, F], dt.bfloat16, tag="hdg")
                nc.scalar.dma_start(hdg_t[:], hdg.ap()[b])
                zg_t = zp.tile([128, n_bt, ZC], dt.float32, tag="zg")
                nc.sync.dma_start(zg_t[:], zg.ap()[b])
                oh_t = ohp.tile([128, n_bt, 128], dt.bfloat16, tag="oh")
                nc.gpsimd.dma_start(oh_t[:], ohd.ap()[b])

                zm = zw.tile([128, n_bt, NH], dt.float32, tag="zm")
                z2 = zw.tile([128, n_bt, NH], dt.float32, tag="z2")
                nc.vector.tensor_add(zm[:], zg_t[:, :, 0:NH], zg_t[:, :, NH:2 * NH])
                nc.vector.tensor_scalar_mul(z2[:], zm[:], NEG)
                nc.vector.tensor_tensor(out=zm[:], in0=zm[:], in1=z2[:], op=mybir.AluOpType.max)
                # exp with per-head expansion via stride-0 read
                ex = xp.tile([128, n_bt, F], dt.bfloat16, tag="ex")
                zexp = zm[:].unsqueeze(-1).to_broadcast([128, n_bt, NH, CW])
                nc.scalar.activation(ex[:].rearrange("p t (h c) -> p t h c", h=NH), zexp,
                                     mybir.ActivationFunctionType.Exp)
                FW = F + NH if layer == 2 else F
                hs = hsp.tile([128, n_bt, FW], dt.bfloat16, tag="hs")
                nc.vector.tensor_mul(hs[:, :, 0:F], hdg_t[:], ex[:])
                if layer == 2:
                    nc.vector.tensor_copy(
                        hs[:, :, F:F + NH],
                        ex[:].rearrange("p t (h c) -> p t h c", h=NH)[:, :, :, 0])

                agg = psa.tile([128, FW], dt.float32, space="PSUM", tag="agg")
                den = None
                if layer == 1:
                    den = psb.tile([128, NH], dt.float32, space="PSUM", tag="den")
                for t in range(n_bt):
                    nc.tensor.matmul(agg[:], oh_t[:, t, :], hs[:, t, :],
                                     start=(t == 0), stop=(t == n_bt - 1))
                    if layer == 1:
                        exs = ex[:].rearrange("p t (h c) -> p t h c", h=NH)[:, t, :, 0]
                        nc.tensor.matmul(den[:], oh_t[:, t, :], exs,
                                         start=(t == 0), stop=(t == n_bt - 1))
                rd = ep.tile([128, NH], dt.float32, tag="rd")
                nc.vector.reciprocal(rd[:], den[:] if layer == 1 else agg[:, F:F + NH])
                if layer == 1:
                    hbf = ep.tile([128, F], dt.bfloat16, tag="hbf")
                    rdx = rd[:].unsqueeze(-1).to_broadcast([128, NH, CW])
                    nc.vector.tensor_tensor(out=hbf[:].rearrange("p (h c) -> p h c", h=NH),
                                            in0=agg[:, 0:F].rearrange("p (h c) -> p h c", h=NH),
                                            in1=rdx, op=mybir.AluOpType.mult)
                    nc.vector.tensor_scalar_max(hbf[:], hbf[:], 0.0)
                    hTp = psc.tile([128, 128], dt.bfloat16, space="PSUM", tag="hT")
                    nc.tensor.transpose(hTp[:], hbf[:], ident[:])
                    hTb = ep.tile([128, 128], dt.bfloat16, tag="hTb")
                    nc.scalar.copy(hTb[:], hTp[:])
                    h2p = psc.tile([66, 128], dt.float32, space="PSUM", tag="h2a")
                    nc.tensor.matmul(h2p[:], w2t[:], hTb[:], start=True, stop=True)
                    h2s = ep.tile([66, 128], dt.float32, tag="h2s")
                    nc.vector.tensor_copy(h2s[:], h2p[:])
                    nc.sync.dma_start(outt.ap()[:, b * 128:(b + 1) * 128], h2s[:])
                else:
                    o2 = ep.tile([128, F], dt.float32, tag="o2")
                    rdx = rd[:].to_broadcast([128, F])
                    nc.vector.tensor_tensor(out=o2[:], in0=agg[:, 0:F], in1=rdx,
                                            op=mybir.AluOpType.mult)
                    nc.sync.dma_start(outt.ap()[b * 128:(b + 1) * 128, :], o2[:])
    nc.compile()
    return nc


# ----------------------------------------------------------------------------
# main entry
# ----------------------------------------------------------------------------

def kernel(x, edge_index, W1, att_src1, att_dst1, b1, W2, att_src2, att_dst2, b2):
    meta = _prep(edge_index)
    W1e = _w1ext(W1, att_src1, att_dst1)
    W2e = _w2ext(W2, att_src2, att_dst2)

    if os.environ.get("GAT_NUMPY"):
        return _run_numpy(x, meta, W1e, W2e)

    from concourse.bass_utils import run_bass_kernel_spmd

    n_bt, NT = meta["n_bt"], meta["NT"]
    old_of_new = meta["old_of_new"]
    real = old_of_new >= 0

    xp = np.zeros((N_PAD, IN_DIM), np.float32)
    xp[real] = np.asarray(x, np.float32)[old_of_new[real]]

    trace = bool(os.environ.get("GAT_TRACE"))
    times = []

    # ---- launch A
    nc_a = _get_cached("A", _build_launch_a)
    in_maps = []
    for c in range(NC):
        sl = slice(c * NODES_PER_CORE, (c + 1) * NODES_PER_CORE)
        in_maps.append({"xT": np.ascontiguousarray(xp[sl].T), "w1e": W1e})
    res = run_bass_kernel_spmd(nc_a, in_maps, core_ids=list(range(NC)), trace=trace)
    times.append(res.exec_time_ns)
    feat = np.concatenate([res.results[c]["featT"].T for c in range(NC)], axis=0)

    hd_bf = feat[:, :128].astype(BF16)
    als, als2x = feat[:, 128:132], feat[:, 132:136]
    ald, ald2x = feat[:, 136:140], feat[:, 140:144]

    eye = np.concatenate([np.eye(128, dtype=np.uint8),
                          np.zeros((1, 128), np.uint8)])

    def _ohot(c):
        dl = meta["dloc_g"][c].astype(np.int64).reshape(N_BLOCKS, meta["n_bt"], 128)
        oh = eye[dl]  # [NB, n_bt, 128p, 128d]
        return np.ascontiguousarray(oh.transpose(0, 2, 1, 3))

    # ---- launch B
    nc_b = _get_cached(("B", n_bt), lambda: _build_edge_launch(1, n_bt))
    in_maps = []
    for c in range(NC):
        sg, dg, v = meta["src_g"][c], meta["dst_g"][c], meta["valid"][c]
        hdgc = hd_bf[sg] * v[:, None]
        z = np.concatenate([als[sg], ald[dg]], axis=1)
        z *= v[:, None]
        in_maps.append({
            "hdg": _pmaj(hdgc, NT), "zg": _pmaj(z.astype(np.float32), NT),
            "ohd": _ohot(c), "w2e": W2e.astype(BF16),
        })
    res = run_bass_kernel_spmd(nc_b, in_maps, core_ids=list(range(NC)), trace=trace)
    times.append(res.exec_time_ns)
    h2a = np.concatenate([res.results[c]["h2a"].T for c in range(NC)], axis=0)

    h2d_bf = h2a[:, :64].astype(BF16)
    als2, ald2 = h2a[:, 64:65], h2a[:, 65:66]

    # ---- launch C
    nc_c = _get_cached(("C", n_bt), lambda: _build_edge_launch(2, n_bt))
    in_maps = []
    for c in range(NC):
        sg, dg, v = meta["src_g"][c], meta["dst_g"][c], meta["valid"][c]
        h2gc = h2d_bf[sg] * v[:, None]
        z = np.concatenate([als2[sg], ald2[dg]], axis=1)
        z *= v[:, None]
        in_maps.append({
            "hdg": _pmaj(h2gc, NT), "zg": _pmaj(z.astype(np.float32), NT),
            "ohd": _ohot(c),
        })
    res = run_bass_kernel_spmd(nc_c, in_maps, core_ids=list(range(NC)), trace=trace)
    times.append(res.exec_time_ns)
    out_pad = np.concatenate([res.results[c]["out2"] for c in range(NC)], axis=0)

    if trace and all(t is not None for t in times):
        kernel.last_exec_ns = sum(times)
        print("per-launch exec ns:", times, "total:", sum(times))

    out = np.zeros((N_NODES, OUT_DIM), np.float32)
    out[old_of_new[real]] = out_pad[real]
    return out


def _get_cached(key, builder):
    if key not in _cache:
        _cache[key] = builder()
    return _cache[key]
